# revision 2
# baseline (speedup 1.0000x reference)
"""Trainium2 Bass kernel for nn_CubEclayr (cubical-complex Euler characteristic curve).

Math: ECC[k] = sum over cells of sign * 1[cell_value <= T_k].  Every cell's value
equals the value of its argmax pixel, so grouping cells by that pixel:
    ECC[k] = sum_p w_p * 1[x_p <= T_k]
with integer weight w_p = 1 - Hcnt - Vcnt + Scnt from local >= comparisons
(tie-breaking provably cannot change the sum since tied pixels share a value).
No cumsum needed: the per-threshold masked count IS the cumulative ECC.

Sharding: batch-parallel, 2 batches (6 planes of 224x224) per core, 8 cores.

Layout per core: padded 2D tensors [128 partitions, 12*226+1 cols]:
12 tiles = 6 planes x 2 row-halves (T0 rows 0..127 | T1 rows 128..223 + 32 pad
rows); each tile has a left/right pad column.  x pads = 2.0 (> all thresholds,
so pad pixels never count regardless of their weight).
"""

import sys
import numpy as np

for _p in ("/opt/trn_rl_repo", "/opt/trn_rl_repo/concourse"):
    if _p not in sys.path:
        sys.path.insert(0, _p)

import dataclasses

import concourse.bass as bass
import concourse.tile as tile
from concourse import mybir
from concourse.bass_utils import run_bass_kernel_spmd

# Exact float32 bits of jnp.linspace(0.0, 1.0, 32) (4 entries differ from
# np.linspace by 1 ulp).
_TSEQ_BITS = np.array([
    0, 1023680776, 1032069384, 1036398988, 1040457992, 1042622794, 1044787596,
    1046952398, 1048846600, 1049929001, 1051011402, 1052093803, 1053176204,
    1054258605, 1055341006, 1056423407, 1057235208, 1057776408, 1058317609,
    1058858810, 1059400010, 1059941210, 1060482411, 1061023612, 1061564812,
    1062106012, 1062647213, 1063188414, 1063729614, 1064270814, 1064812015,
    1065353216], dtype=np.uint32)
TSEQ = _TSEQ_BITS.view(np.float32)

B, C, H, W = 16, 3, 224, 224
N_CORES = 8
B_PER_CORE = B // N_CORES           # 2
PLANES = B_PER_CORE * C             # 6
TILE_W = 226                        # 1 pad + 224 + 1 pad
N_TILES = 2 * PLANES                # T0 halves then T1 halves
FREE = N_TILES * TILE_W             # 2712
FREEX = FREE + 1                    # +1 init column for shifted full-FD reads
T1_OFF = PLANES * TILE_W            # 1356
F32 = mybir.dt.float32
ALU = mybir.AluOpType
SENTINEL = 2.0


def _split_drain_waits(nc, max_waits=1):
    """walrus codegen caps sem waits per instruction at 1 for CTRL/DMA
    pseudo-ops.  Hoist excess waits onto same-engine NOPs placed just before
    (engine FIFO order makes them execute first)."""
    for f in nc.m.functions:
        for bb in f.blocks:
            new_list = []
            for ins in bb.instructions:
                si = ins.sync_info
                if si and len(si.on_wait) > max_waits:
                    waits = list(si.on_wait)
                    extra, keep = waits[:-max_waits], waits[-max_waits:]
                    for ci, w in enumerate(extra):
                        new_list.append(mybir.InstNoOp(
                            name=f"{ins.name}_w{ci}", engine=ins.engine,
                            sync_info=mybir.SyncInfo(on_wait=[w], on_update=[]),
                        ))
                    ins.sync_info = mybir.SyncInfo(
                        on_wait=keep, on_update=list(si.on_update))
                    new_list.append(ins)
                else:
                    new_list.append(ins)
            bb.instructions[:] = new_list


def build_program():
    nc = bass.Bass()
    x_d = nc.dram_tensor("x", [B_PER_CORE, C, H, W], F32, kind="ExternalInput")
    y_d = nc.dram_tensor("y", [128, PLANES * 32], F32, kind="ExternalOutput")

    with tile.TileContext(nc) as tc:
        with tc.tile_pool(name="main", bufs=1) as pool:
            xt = pool.tile([128, FREEX], F32)    # x values (padded layout)
            xs = pool.tile([128, FREEX], F32)    # x shifted up one row
            hm = pool.tile([128, FREEX], F32)    # max(x, x-right)
            hs = pool.tile([128, FREEX], F32)    # hm shifted up one row
            g = pool.tile([128, FREEX], F32)     # 1[x >= x-right]
            eb = pool.tile([128, FREEX], F32)    # 1[x >= x-below]
            tt = pool.tile([128, FREEX], F32)    # 1[hm >= hs]
            tu = pool.tile([128, FREEX], F32)    # tt shifted down one row
            ebd = pool.tile([128, FREEX], F32)   # eb shifted down one row
            dd = pool.tile([128, FREEX], F32)    # tt - tu
            gc = pool.tile([128, FREEX], F32)    # 1 - g
            ab = pool.tile([128, FREEX], F32)    # A then B term
            wt = pool.tile([128, FREEX], F32)    # weight accumulator
            scratch = pool.tile([128, 2 * TILE_W], F32)
            acc = pool.tile([128, PLANES * 32], F32)

            # ---- load x into padded layout; pads/sentinels = 2.0 ----
            nc.vector.memset(xt[:], SENTINEL)
            src_t0 = x_d[:, :, 0:128, :].rearrange("b c r w -> r (b c) w")
            dst = xt[:, 0:FREE].rearrange("p (t c) -> p t c", c=TILE_W)
            nc.sync.dma_start(out=dst[:, 0:PLANES, 1:225], in_=src_t0)
            src_t1 = x_d[:, :, 128:224, :].rearrange("b c r w -> r (b c) w")
            nc.sync.dma_start(out=dst[0:96, PLANES:N_TILES, 1:225], in_=src_t1)

            # ---- xs = x shifted up one row (row r+1), sentinel elsewhere ----
            nc.vector.memset(xs[:], SENTINEL)
            # T0 rows 0..126 <- x rows 1..127
            nc.sync.dma_start(out=xs[0:127, 0:T1_OFF], in_=xt[1:128, 0:T1_OFF])
            # T0 row 127 <- x row 128 (= T1 partition 0)
            nc.sync.dma_start(out=xs[127:128, 0:T1_OFF], in_=xt[0:1, T1_OFF:FREE])
            # T1 rows 0..94 <- x rows 129..223
            nc.sync.dma_start(out=xs[0:95, T1_OFF:FREE], in_=xt[1:96, T1_OFF:FREE])
            # (xs T1 partitions 95.. stay 2.0: row 223 has no below-neighbor)

            # ---- elementwise prep (all full-FD ops on [128, 2712]) ----
            x0 = xt[:, 0:FREE]
            x1 = xt[:, 1:FREEX]
            s0 = xs[:, 0:FREE]
            s1 = xs[:, 1:FREEX]
            nc.vector.tensor_tensor(out=hm[:, 0:FREE], in0=x0, in1=x1, op=ALU.max)
            nc.vector.tensor_tensor(out=hs[:, 0:FREE], in0=s0, in1=s1, op=ALU.max)
            nc.vector.tensor_tensor(out=g[:, 0:FREE], in0=x0, in1=x1, op=ALU.is_ge)
            nc.vector.tensor_tensor(out=eb[:, 0:FREE], in0=x0, in1=s0, op=ALU.is_ge)
            nc.vector.tensor_tensor(out=tt[:, 0:FREE], in0=hm[:, 0:FREE],
                                    in1=hs[:, 0:FREE], op=ALU.is_ge)
            # init extra col of g/tt chains read by shifted views
            nc.vector.memset(g[:, FREE:FREEX], 1.0)
            nc.vector.memset(tt[:, FREE:FREEX], 0.0)

            # ---- row-down shifts (tu = t(r-1), ebd = eb(r-1)); phantom = 1 ----
            nc.vector.memset(tu[:], 1.0)
            nc.sync.dma_start(out=tu[1:128, 0:T1_OFF], in_=tt[0:127, 0:T1_OFF])
            nc.sync.dma_start(out=tu[0:1, T1_OFF:FREE], in_=tt[127:128, 0:T1_OFF])
            nc.sync.dma_start(out=tu[1:96, T1_OFF:FREE], in_=tt[0:95, T1_OFF:FREE])
            nc.vector.memset(ebd[:], 1.0)
            nc.sync.dma_start(out=ebd[1:128, 0:T1_OFF], in_=eb[0:127, 0:T1_OFF])
            nc.sync.dma_start(out=ebd[0:1, T1_OFF:FREE], in_=eb[127:128, 0:T1_OFF])
            nc.sync.dma_start(out=ebd[1:96, T1_OFF:FREE], in_=eb[0:95, T1_OFF:FREE])

            # ---- weight w = 1 - Hcnt - Vcnt + Scnt ----
            # dd = t - tu
            nc.vector.tensor_tensor(out=dd[:, 0:FREE], in0=tt[:, 0:FREE],
                                    in1=tu[:, 0:FREE], op=ALU.subtract)
            nc.vector.memset(dd[:, FREE:FREEX], 0.0)
            # gc = 1 - g
            nc.vector.tensor_scalar(out=gc[:, 0:FREE], in0=g[:, 0:FREE],
                                    scalar1=-1.0, scalar2=1.0,
                                    op0=ALU.mult, op1=ALU.add)
            nc.vector.memset(gc[:, FREE:FREEX], 0.0)
            # A = (dd + 1) * g
            nc.vector.scalar_tensor_tensor(
                out=ab[:, 0:FREE], in0=dd[:, 0:FREE], scalar=1.0,
                in1=g[:, 0:FREE], op0=ALU.add, op1=ALU.mult)
            # w = ebd - eb
            nc.vector.tensor_tensor(out=wt[:, 0:FREE], in0=ebd[:, 0:FREE],
                                    in1=eb[:, 0:FREE], op=ALU.subtract)
            # w += gL  (out cols 1.. read g cols 0..)
            nc.vector.tensor_tensor(out=wt[:, 1:FREEX], in0=wt[:, 1:FREEX],
                                    in1=g[:, 0:FREE], op=ALU.add)
            # w -= g
            nc.vector.tensor_tensor(out=wt[:, 0:FREE], in0=wt[:, 0:FREE],
                                    in1=g[:, 0:FREE], op=ALU.subtract)
            # w += A
            nc.vector.tensor_tensor(out=wt[:, 0:FREE], in0=wt[:, 0:FREE],
                                    in1=ab[:, 0:FREE], op=ALU.add)
            # B = (dd_L + 1) * gc_L  (into ab; col 0 of ab left stale but finite)
            nc.vector.scalar_tensor_tensor(
                out=ab[:, 1:FREEX], in0=dd[:, 0:FREE], scalar=1.0,
                in1=gc[:, 0:FREE], op0=ALU.add, op1=ALU.mult)
            # w = (B - 1) + w
            nc.vector.scalar_tensor_tensor(
                out=wt[:, 0:FREE], in0=ab[:, 0:FREE], scalar=-1.0,
                in1=wt[:, 0:FREE], op0=ALU.add, op1=ALU.add)

            # ---- histogram: acc[p, plane*32+k] = sum_f w * 1[x <= T_k] ----
            sc_view = scratch[:].rearrange("p (h c) -> p h c", c=TILE_W)
            for plane in range(PLANES):
                xv = dst[:, plane:N_TILES:PLANES, :]    # [128, 2, 226]
                wv = wt[:, 0:FREE].rearrange(
                    "p (t c) -> p t c", c=TILE_W)[:, plane:N_TILES:PLANES, :]
                for k in range(32):
                    nc.vector.scalar_tensor_tensor(
                        out=sc_view, in0=xv, scalar=float(TSEQ[k]), in1=wv,
                        op0=ALU.is_le, op1=ALU.mult,
                        accum_out=acc[:, plane * 32 + k: plane * 32 + k + 1])

            nc.sync.dma_start(out=y_d[:], in_=acc[:])

    _split_drain_waits(nc)
    return nc


_NC_CACHE = None


def _get_nc():
    global _NC_CACHE
    if _NC_CACHE is None:
        _NC_CACHE = build_program()
    return _NC_CACHE


def kernel(x: np.ndarray) -> np.ndarray:
    x = np.ascontiguousarray(x, dtype=np.float32)
    assert x.shape == (B, C, H, W)
    nc = _get_nc()
    in_maps = [{"x": x[i * B_PER_CORE:(i + 1) * B_PER_CORE]} for i in range(N_CORES)]
    res = run_bass_kernel_spmd(nc, in_maps, core_ids=list(range(N_CORES)))
    out = np.empty((B, C * 32), dtype=np.float32)
    for i in range(N_CORES):
        acc = np.asarray(res.results[i]["y"], dtype=np.float64)  # [128, 192]
        sums = acc.sum(axis=0)                                   # [192]
        out[i * B_PER_CORE:(i + 1) * B_PER_CORE] = (
            sums.reshape(B_PER_CORE, C * 32).astype(np.float32))
    return out


if __name__ == "__main__":
    rng = np.random.default_rng(0)
    xtest = rng.random((B, C, H, W), dtype=np.float32)
    out = kernel(xtest)
    print("kernel output shape:", out.shape)
    print(out[0, :8])
